# revision 18
# baseline (speedup 1.0000x reference)
"""Bass/Trainium2 kernel for nn_KernelEdges (gnn_message_passing).

Computes A = exp((g_i + g_j - 2*Xf@Xf.T)/sigma^2) with zeroed diagonal,
broadcast to all B batch slots, where Xf = X.transpose(1,0,2).reshape(N, B*d).

Sharding: rows of the NxN pairwise matrix are split across 8 NeuronCores
(256 rows each).  The batch dim of the output is a pure replication of the
same [N, N] matrix, so each core writes only its unique [N/8, N] tile and
the host broadcasts to the B batch slots (as the reference itself does).

Each core receives a column-ROLLED copy of XT = Xf.T [B*d, N] so that its
own 256 columns sit at rolled positions 0..255; the matmul LHS (stationary
operand) is then a fixed slice of the streamed xt tile and no separate
lhst input is needed.  The host un-rolls the output columns after gather.

Math decomposition (exp(a+b) = exp(a)*exp(b)):
  psum[m, n] = sum_q xt_q[:, m_cols].T @ xt_q[:, n_block]   (Gram matrix)
  t = exp(-2/sigma^2 * psum + g_i/sigma^2)                  (ACT, bias/row)
  A = t * e_j,  e_j = exp(g_j/sigma^2)                      (DVE, row bcast)
The e_j row factor replaces rank-1 g_j matmuls (which cost ~5us of PE
column-streaming); e_j is replicated across partitions once by a gpsimd
partition_broadcast issued FIRST on that queue (ucode instructions stall
on the queue's outstanding DMAs, so nothing else may precede them).

DMA shape discipline: transfers are row-descriptor-rate limited (~6.5ns
per partition-row core-wide) until rows reach ~4KB, so every stream uses
>=2KB rows: xt streams as 8 half-tiles [128, 1024] bf16 (h-major, so the
h0 psum chains finish and their ACT+mul+store launch while h1 is still
loading), and the output leaves as 4 pieces [128, 1024] bf16.

Engine/queue budget (a DMA trigger costs ~600ns of issuing-engine time;
only gpsimd/sync/scalar can issue DMAs):
  tensor: 32 Gram matmuls only
  scalar: bias + 2 xt triggers, then 4 wide ACTs
  vector: 4 wide e_j multiplies (cannot DMA)
  gpsimd: ej trigger + partition_broadcast halves + 2 xt triggers
  sync:   4 xt triggers, then 4 output triggers

The diagonal is zeroed on the host (2K elements) after the gather.
"""

import numpy as np

B, N, D = 8, 2048, 64
NCORES = 8
R = N // NCORES          # 256 rows per core
KD = B * D               # 512 contraction dim
NB = 512                 # n-block (one PSUM bank of fp32)
NH = 2                   # column halves (streaming granularity)
HW = N // NH             # 1024 cols per half
NMT = R // 128           # 2 m-tiles per core
NQ = KD // 128           # 4 k-tiles

MM_MODE = "bf16"         # matmul operand dtype ("bf16" | "f32r")
OUT_BF16 = True          # store A as bf16, upcast on host
# "rank1": fold g_j into PSUM via rank-1 matmuls (~8 extra PE matmuls).
# "pb" (e_j row via gpsimd partition_broadcast + DVE multiply) is kept for
# reference but measured WORSE: the gpsimd ucode instruction behaves as a
# barrier against all prior work, running only after the last matmul and
# serializing a ~13us tail of multiplies + stores.
EJ_MODE = "rank1"

# DMA model (measured): each HWDGE ring moves ~80-85 G *elements*/s
# (dtype-independent!), with ~2.1us trigger->data pipeline-fill latency;
# the 3 legal rings (sync/scalar/gpsimd) run in parallel.  Two tricks:
# all transfers are bitcast to uint32 (2 bf16 packed per element, halving
# wire elements), and the 8 xt pieces round-robin over the rings so they
# complete roughly in PE consumption order.  scalar's ring starts late
# (ACT table load) so it carries only bias + the last piece.
PIECE_QUEUE = {
    0: "sync", 1: "sync", 2: "sync", 6: "sync",
    3: "gpsimd", 4: "gpsimd", 5: "gpsimd",
    7: "scalar",
}
NJUNK = 11               # PE-warmup matmuls: sized so the junk stream ends
                         # right when the grow data lands (~10.0us) — a gap
                         # between junk and the real stream resets the PE
                         # p-state ramp and costs ~2us of mid-speed matmuls


def _build_program(inv_s2):
    import concourse.bass as bass
    import concourse.tile as tile
    from concourse import bacc, mybir

    f32 = mybir.dt.float32
    mm_dt = mybir.dt.bfloat16 if MM_MODE == "bf16" else mybir.dt.float32r
    out_dt = mybir.dt.bfloat16 if OUT_BF16 else f32

    nc = bacc.Bacc(
        "TRN2", target_bir_lowering=False, debug=False, num_devices=NCORES
    )

    GK = 2 if MM_MODE == "bf16" else 1  # g carried as hi+lo rows in bf16

    # xt pre-tiled on host: piece (h, q) = rows (h*NQ+q)*128..+128, fully
    # contiguous in DRAM, 2KB rows
    xt_d = nc.dram_tensor(
        "xt", [NH * NQ * 128, HW], mm_dt, kind="ExternalInput"
    ).ap()
    bias_d = nc.dram_tensor("bias", [128, NMT], f32, kind="ExternalInput").ap()
    if EJ_MODE == "pb":
        ej_d = nc.dram_tensor("ej", [1, N], f32, kind="ExternalInput").ap()
    else:
        grow_d = nc.dram_tensor(
            "grow", [GK, N], mm_dt, kind="ExternalInput"
        ).ap()
    # out piece (mt, h) at rows (mt*NH+h)*128..+128, contiguous, 2KB rows
    out_d = nc.dram_tensor(
        "out", [NMT * NH * 128, HW], out_dt, kind="ExternalOutput"
    ).ap()

    with tile.TileContext(nc) as tc:
        with (
            tc.tile_pool(name="persist", bufs=1) as persist,
            tc.tile_pool(name="apool", bufs=1) as apool,
            tc.tile_pool(name="psum", bufs=1, space="PSUM") as pspool,
        ):
            # ---- gpsimd: ej load + partition broadcast FIRST ----
            if EJ_MODE == "pb":
                ej_sb = persist.tile([1, N], f32, name="ej")
                nc.gpsimd.dma_start(ej_sb[:], ej_d[:])
                ejr_sb = persist.tile([128, N], f32, name="ejr")
                for h in range(NH):
                    sl = slice(h * HW, (h + 1) * HW)
                    nc.gpsimd.partition_broadcast(ejr_sb[:, sl], ej_sb[:, sl])
            else:
                # memsets first: they are pure engine ops, so the junk
                # warmup matmuls can start before any DMA data lands
                junk_sb = persist.tile([128, 256], mm_dt, name="junk")
                nc.gpsimd.memset(junk_sb[:].bitcast(mybir.dt.uint16), 0)
                neg_half = persist.tile([GK, 128], mm_dt, name="neg_half")
                if MM_MODE == "bf16":
                    nc.gpsimd.memset(
                        neg_half[:].bitcast(mybir.dt.uint16), 0xBF00
                    )
                else:
                    nc.gpsimd.memset(
                        neg_half[:].bitcast(mybir.dt.uint32), 0xBF000000
                    )
                # grow rides the scalar ring FIRST: the seeds gate the real
                # PE stream, and scalar's ring is otherwise nearly idle
                grow_sb = persist.tile([GK, N], mm_dt, name="grow")
                nc.scalar.dma_start(grow_sb[:], grow_d[:])

            bias_sb = persist.tile([128, NMT], f32, name="bias")
            nc.scalar.dma_start(bias_sb[:], bias_d[:])

            # ---- xt half-tile loads, h-major, spread across 3 DGE rings --
            xt_sb = [
                persist.tile([128, N], mm_dt, name=f"xt{q}")
                for q in range(NQ)
            ]
            engines = {
                "sync": nc.sync, "scalar": nc.scalar, "gpsimd": nc.gpsimd
            }
            u32 = mybir.dt.uint32
            for idx in range(NH * NQ):
                h, q = divmod(idx, NQ)
                row0 = idx * 128
                engines[PIECE_QUEUE[idx]].dma_start(
                    xt_sb[q][:, h * HW:(h + 1) * HW].bitcast(u32),
                    xt_d[row0:row0 + 128, :].bitcast(u32),
                )

            # ---- compute + store ----
            # 4 psum tiles of two banks each; chain (mt, h) spans both bank
            # halves so one wide ACT/mul/store covers it
            ps = {
                (mt, h): pspool.tile([128, HW], f32, name=f"ps{mt}{h}")
                for h in range(NH) for mt in range(NMT)
            }
            if EJ_MODE == "pb":
                a_tmp = {
                    mt: apool.tile([128, N], f32, name=f"t{mt}")
                    for mt in range(NMT)
                }
            a_sb = {
                mt: apool.tile([128, N], out_dt, name=f"a{mt}")
                for mt in range(NMT)
            }
            # matmul order matches piece arrival order (PE is in-order);
            # within (h, q): mt-grouped so chain (mt0, h) stops before
            # (mt1, h) and the wide ACT pipeline starts earliest.
            # LHS is the core's own 256 rolled columns, a slice of piece
            # (h=0, q) which is always already resident.
            # rank-1 seeds interleave per-half: they depend only on grow
            # (arrives first), so h0's seeds warm the PE before piece 0
            # lands and h1's seeds hide in the stream
            # PE p-state warmup: junk matmuls into a bank that the first
            # real chain re-seeds with start=True (which resets PSUM), so
            # the garbage never escapes.  Keeps the PE continuously busy
            # from ~6.8us; the 2.4GHz ramp needs ~3us of busy time.
            if EJ_MODE == "rank1":
                for _ in range(NJUNK):
                    nc.tensor.matmul(
                        ps[0, 0][:, 0:256],
                        junk_sb[:, 0:128],
                        junk_sb[:],
                        start=True,
                        stop=True,
                    )
            for h in range(NH):
                if EJ_MODE == "rank1":
                    for mt in range(NMT):
                        for nbh in range(2):
                            nc.tensor.matmul(
                                ps[mt, h][:, nbh * NB:(nbh + 1) * NB],
                                neg_half[:],
                                grow_sb[
                                    :, h * HW + nbh * NB:
                                    h * HW + (nbh + 1) * NB
                                ],
                                start=True,
                                stop=False,
                            )
                for q in range(NQ):
                    for mt in range(NMT):
                        for nbh in range(2):
                            nc.tensor.matmul(
                                ps[mt, h][:, nbh * NB:(nbh + 1) * NB],
                                xt_sb[q][:, mt * 128:(mt + 1) * 128],
                                xt_sb[q][
                                    :, h * HW + nbh * NB:
                                    h * HW + (nbh + 1) * NB
                                ],
                                start=(q == 0 and EJ_MODE == "pb"),
                                stop=(q == NQ - 1),
                            )
            # wide ACT + e_j multiply + store chase the chains in stop order
            for h in range(NH):
                for mt in range(NMT):
                    sl = slice(h * HW, (h + 1) * HW)
                    row0 = (mt * NH + h) * 128
                    if EJ_MODE == "pb":
                        nc.scalar.activation(
                            a_tmp[mt][:, sl],
                            ps[mt, h][:],
                            mybir.ActivationFunctionType.Exp,
                            bias=bias_sb[:, mt:mt + 1],
                            scale=-2.0 * inv_s2,
                        )
                        nc.vector.tensor_mul(
                            a_sb[mt][:, sl], a_tmp[mt][:, sl], ejr_sb[:, sl]
                        )
                    else:
                        nc.scalar.activation(
                            a_sb[mt][:, sl],
                            ps[mt, h][:],
                            mybir.ActivationFunctionType.Exp,
                            bias=bias_sb[:, mt:mt + 1],
                            scale=-2.0 * inv_s2,
                        )
                    # stores alternate between the sync and gpsimd rings
                    # (both idle by now) and ride the wire as uint32
                    st_eng = nc.sync if (h * NMT + mt) % 2 == 0 else nc.gpsimd
                    st_eng.dma_start(
                        out_d[row0:row0 + 128, :].bitcast(u32),
                        a_sb[mt][:, sl].bitcast(u32),
                    )

    nc.compile()
    return nc


def _prepare(X, log_sigma):
    """Host prep: returns (inv_s2, in_maps) for run_bass_kernel_spmd."""
    import ml_dtypes

    X = np.ascontiguousarray(X, dtype=np.float32)
    assert X.shape == (B, N, D), X.shape

    sigma = float(np.exp(np.float32(log_sigma)))
    inv_s2 = 1.0 / (sigma * sigma)

    # XT[b*D+f, n] = X[b, n, f]
    XT = np.ascontiguousarray(X.transpose(0, 2, 1).reshape(KD, N))
    g = np.einsum("kn,kn->n", XT, XT).astype(np.float32)  # [N]

    mm_np = ml_dtypes.bfloat16 if MM_MODE == "bf16" else np.float32
    XTm = XT.astype(mm_np)

    in_maps = []
    for c in range(NCORES):
        r0 = c * R
        # roll columns so this core's own block is at rolled cols 0..R-1
        Xr = np.roll(XTm, -r0, axis=1)
        # pre-tile: piece (h, q) contiguous -> [NH*NQ*128, HW]
        xt_t = np.ascontiguousarray(
            Xr.reshape(NQ, 128, NH, HW).transpose(2, 0, 1, 3)
        ).reshape(NH * NQ * 128, HW)

        gr = np.roll(g, -r0)
        bias_np = np.empty((128, NMT), dtype=np.float32)
        for mt in range(NMT):
            bias_np[:, mt] = g[r0 + mt * 128: r0 + (mt + 1) * 128] * inv_s2
        im = {"xt": xt_t, "bias": bias_np}
        if EJ_MODE == "pb":
            im["ej"] = np.ascontiguousarray(
                np.exp(gr * inv_s2, dtype=np.float32)[None, :]
            )
        else:
            if MM_MODE == "bf16":
                g_hi = gr.astype(ml_dtypes.bfloat16)
                g_lo = (gr - g_hi.astype(np.float32)).astype(
                    ml_dtypes.bfloat16
                )
                im["grow"] = np.ascontiguousarray(np.stack([g_hi, g_lo]))
            else:
                im["grow"] = np.ascontiguousarray(gr[None, :])
        in_maps.append(im)
    return inv_s2, in_maps


def kernel(X, log_sigma):
    from concourse.bass_utils import run_bass_kernel_spmd

    inv_s2, in_maps = _prepare(X, log_sigma)
    nc = _build_program(inv_s2)
    res = run_bass_kernel_spmd(nc, in_maps, list(range(NCORES)))

    A = np.empty((N, N), dtype=np.float32)
    for c in range(NCORES):
        r0 = c * R
        t = np.asarray(res.results[c]["out"])
        # un-tile: [NMT*NH*128, HW] -> [R, N] (still column-rolled)
        t = t.reshape(NMT, NH, 128, HW).transpose(0, 2, 1, 3).reshape(R, N)
        # un-roll columns back to global positions
        A[r0:r0 + R, :] = np.roll(t.astype(np.float32), r0, axis=1)
    idx = np.arange(N)
    A[idx, idx] = 0.0
    out = np.empty((B, N, N), dtype=np.float32)
    out[:] = A[None, :, :]
    return out
